# revision 20
# baseline (speedup 1.0000x reference)
"""Trainium2 Bass kernel for nn_KernelGraphAttentionNetwork.

Strategy (8 NeuronCores):
  Ship each core a UNIQUE (256, 768) fp8-e4m3 t-major slice of the RAW
  sentence reps (cores 0-3: batch 0 query-groups 0-3, cores 4-7:
  batch 1), cast on host via a bias-compensated 64K lookup table
  straight into the shipping buffer. The per-core consts rows carry
  the selector weights and the f32 inverse norms as raw bytes
  (bitcast to f32 on device). On device: AllGather over NeuronLink
  gives every core all 2048 columns (both batches); PE identity
  transposes build the D-major lhsT/rhs tiles; then each core
  computes the edge-kernel logits for its own 256 query tokens
  against all 32 gathered sentences:
      sim   = (x_own^T @ x_all) * inv_i * inv_j  (PE + DVE/ScalarE)
      rbf_k = exp(-(sim-mu_k)^2/(2 s_k^2))       (ScalarE Square+Exp)
      pool  = sum_q rbf_k                        (VectorE reduce over T2)
      logit = sum_k ln(clip(pool)) * w_sel       (ScalarE Ln + VectorE)
  Host keeps the 16 columns of the core's own batch and finishes the
  small coupled tail in f32 numpy: T1-softmax, z_hat einsum, gating
  MLP, beta softmax over S1, label head, node kernel, rationale (the
  logits-independent parts run while the device round-trip is in
  flight; the result is fetched via a pre-queued async D2H).

  All heavy one-time setup (jax/concourse import, Bass build, NEFF
  compile, executable load, axon warm-up) happens at module import so
  kernel() itself is a single warm dispatch. The device kernel is
  partition-id free (uniform SPMD program; per-core behavior comes
  only from per-core input data), executed on cores 0-7 through the
  same bass_exec/PJRT path run_bass_kernel_spmd uses under axon.
"""

import numpy as np

KERNEL = 11
B, S, T, D = 2, 16, 64, 768
SG = 2 * S           # 32 gathered sentences (both batches)
EPS = 1e-6
CLAMP_MIN = 1e-6
N_CORES = 8
NK = KERNEL - 1      # k=0 (sigma=1e-3) is constant over T1 -> softmax-invariant


def _kernel_mus(n):
    mus = [1.0]
    if n == 1:
        return mus
    b = 2.0 / (n - 1)
    mus.append(1.0 - b / 2.0)
    for i in range(1, n - 1):
        mus.append(mus[i] - b)
    return mus


MU = np.asarray(_kernel_mus(KERNEL), dtype=np.float64)
SIGMA = np.asarray([0.001] + [0.1] * (KERNEL - 1), dtype=np.float64)
MU32 = MU.astype(np.float32)
SIG32 = SIGMA.astype(np.float32)

_STATE = {}
LAST_RESULTS = None


def _build_sharded():
    import jax
    from jax.sharding import Mesh, PartitionSpec as P
    from concourse.bass2jax import bass_jit, bass_shard_map
    from concourse import mybir, masks
    import concourse.bass as bass
    import concourse.tile as tile

    @bass_jit(trn_type="TRN2", enable_asserts=False, num_devices=N_CORES)
    def _edge_logits_ag(nc, rpart):
        f32 = mybir.dt.float32
        f8 = mybir.dt.float8e4
        AF = mybir.ActivationFunctionType
        bf16 = mybir.dt.bfloat16
        logits_out = nc.dram_tensor(
            "logits_out", [2, 128, SG], bf16, kind="ExternalOutput"
        )
        rpart = rpart[:]

        with tile.TileContext(nc) as tc:
            with (
                tc.tile_pool(name="dram", bufs=1, space="DRAM") as dram_pool,
                tc.tile_pool(name="rt", bufs=1) as rt_pool,
                tc.tile_pool(name="ri", bufs=1) as ri_pool,
                tc.tile_pool(name="stg", bufs=4) as stg_pool,
                tc.tile_pool(name="cst", bufs=1) as cst_pool,
                tc.tile_pool(name="psum", bufs=1, space="PSUM") as psum_pool,
                tc.tile_pool(name="ptr", bufs=4, space="PSUM") as ptr_pool,
                tc.tile_pool(name="work", bufs=4) as work_pool,
                tc.tile_pool(name="pacc", bufs=2) as pacc_pool,
                tc.tile_pool(name="outs", bufs=2) as out_pool,
            ):
                # input is t-major (256 token rows x 768 dims); all-gather
                # the raw slices, then build the D-major lhsT/rhs tiles on
                # the PE via identity transposes (host ships natural layout
                # and skips its 2ms strided transpose pass).
                in_bounce = dram_pool.tile([256, D], f8, tag="inb")
                ag_out = dram_pool.tile([N_CORES * 256, D], f8, tag="agb")
                nc.gpsimd.dma_start(in_bounce[:], rpart[:256, :])
                nc.gpsimd.collective_compute(
                    "AllGather",
                    mybir.AluOpType.bypass,
                    replica_groups=[list(range(N_CORES))],
                    ins=[in_bounce.opt()],
                    outs=[ag_out.opt()],
                )

                ident = cst_pool.tile([128, 128], f8, tag="ident")
                masks.make_identity(nc, ident[:])

                ri = []
                rt = []
                for dc in range(6):
                    ri_t = ri_pool.tile([128, 256], f8, tag=f"ri{dc}")
                    ri.append(ri_t)
                    rt_t = rt_pool.tile([128, SG * T], f8, tag=f"rt{dc}")
                    rt.append(rt_t)
                agp = ag_out[:]

                def _load_transposed(src_rows, col_base, dst_tiles):
                    # src_rows: DRAM AP (128 t-rows, 768); writes column block
                    # [col_base:col_base+128] of each dst_tiles[dc].
                    st = stg_pool.tile([128, D], f8, tag="stage")
                    nc.sync.dma_start(out=st, in_=src_rows)
                    for dc in range(6):
                        # fp8 PE transpose requires output element step 2
                        ptr = ptr_pool.tile([128, 256], f8, tag="tr")
                        pv = ptr[:, 0:256:2]
                        nc.tensor.transpose(
                            pv, st[:, dc * 128 : (dc + 1) * 128], ident[:]
                        )
                        nc.any.tensor_copy(
                            dst_tiles[dc][:, col_base : col_base + 128], pv
                        )

                for half in range(2):
                    _load_transposed(
                        rpart[half * 128 : (half + 1) * 128, :], half * 128, ri
                    )
                for g in range(N_CORES):
                    for half in range(2):
                        _load_transposed(
                            agp[g * 256 + half * 128 : g * 256 + (half + 1) * 128, :],
                            g * 256 + half * 128,
                            rt,
                        )

                # consts ride as raw bytes in rows D..D+42 of rpart
                # (wk f32[330] | inv_own f32[256] | inv_all f32[2048]):
                # broadcast the fp8 bytes to all partitions, then bitcast
                # the SBUF tiles to f32 at the consumers.
                NCONST = SG * NK + NK
                cbase = rpart.offset + 256 * D
                csb = cst_pool.tile([128, 4 * NCONST], f8)
                nc.sync.dma_start(
                    out=csb,
                    in_=bass.AP(tensor=rpart.tensor, offset=cbase,
                                ap=[[0, 128], [1, 4 * NCONST]]),
                )
                cf = csb[:].bitcast(f32)             # (128, NCONST)
                wsel_b = cf[:, : SG * NK]
                negmu_b = cf[:, SG * NK :]
                invi = []                            # own 1/|x| per partition
                for ip in range(2):
                    t_ = cst_pool.tile([128, 4], f8, tag=f"invi{ip}")
                    nc.sync.dma_start(
                        out=t_,
                        in_=bass.AP(tensor=rpart.tensor,
                                    offset=cbase + 4 * NCONST + ip * 512,
                                    ap=[[4, 128], [1, 4]]),
                    )
                    invi.append(t_[:].bitcast(f32))  # (128, 1)
                invj_t = cst_pool.tile([128, 4 * SG * T], f8)
                nc.sync.dma_start(
                    out=invj_t,
                    in_=bass.AP(tensor=rpart.tensor,
                                offset=cbase + 4 * NCONST + 1024,
                                ap=[[0, 128], [1, 4 * SG * T]]),
                )
                invj = invj_t[:].bitcast(f32)        # (128, 2048) bcast

                NCH = (SG * T) // 512
                for ip in range(2):
                    sim_sb = work_pool.tile([128, SG * T], f32, tag="sim_sb")
                    for nch in range(NCH):
                        ps = psum_pool.tile([128, 512], f32, tag=f"sim{nch}")
                        for dc in range(6):
                            nc.tensor.matmul(
                                ps,
                                lhsT=ri[dc][:, ip * 128 : (ip + 1) * 128],
                                rhs=rt[dc][:, nch * 512 : (nch + 1) * 512],
                                start=(dc == 0),
                                stop=(dc == 5),
                            )
                        # fold 1/|x_j| here; 1/|x_i| rides the Square scale
                        nc.vector.tensor_mul(
                            out=sim_sb[:, nch * 512 : (nch + 1) * 512],
                            in0=ps,
                            in1=invj[:, nch * 512 : (nch + 1) * 512],
                        )

                    poolk = pacc_pool.tile([128, SG, NK], f32)
                    for kk in range(NK):
                        k = kk + 1
                        alpha = float(0.5 / (SIGMA[k] ** 2))
                        d2 = work_pool.tile([128, SG * T], f32, tag="d2")
                        nc.scalar.activation(
                            out=d2,
                            in_=sim_sb,
                            func=AF.Square,
                            bias=negmu_b[:, kk : kk + 1],
                            scale=invi[ip],
                        )
                        e = work_pool.tile([128, SG * T], f32, tag="e")
                        nc.scalar.activation(out=e, in_=d2, func=AF.Exp, scale=-alpha)
                        nc.vector.reduce_sum(
                            out=poolk[:, :, kk : kk + 1],
                            in_=e.rearrange("p (j q) -> p j q", q=T),
                            axis=mybir.AxisListType.X,
                        )

                    pkf = poolk.rearrange("p j k -> p (j k)")
                    nc.vector.tensor_scalar_max(out=pkf, in0=pkf, scalar1=CLAMP_MIN)
                    ke = work_pool.tile([128, SG * NK], f32, tag="ke")
                    nc.scalar.activation(out=ke, in_=pkf, func=AF.Ln)
                    nc.vector.tensor_mul(out=ke, in0=ke, in1=wsel_b)
                    lg = out_pool.tile([128, SG], bf16, tag="lg")
                    with nc.allow_low_precision(
                        reason="logits returned bf16; 10-term sum, tol 2e-2"
                    ):
                        nc.vector.reduce_sum(
                            out=lg,
                            in_=ke.rearrange("p (j k) -> p j k", k=NK),
                            axis=mybir.AxisListType.X,
                        )
                    nc.sync.dma_start(out=logits_out[:][ip], in_=lg)
        return (logits_out,)

    devices = jax.devices()[:N_CORES]
    mesh = Mesh(np.asarray(devices), ("core",))
    sharded = bass_shard_map(
        _edge_logits_ag,
        mesh=mesh,
        in_specs=(P("core"),),
        out_specs=(P("core"),),
    )
    return sharded


def _setup():
    try:
        import ml_dtypes

        sharded = _build_sharded()
        f8 = ml_dtypes.float8_e4m3
        dummy_rp = np.zeros((N_CORES * 270, D), f8)
        out = sharded(dummy_rp)
        np.asarray(out[0])  # force compile + load + one execution
        _STATE["sharded"] = sharded
        _STATE["f8"] = f8
        # f32 -> e4m3 via truncate-to-bf16 + 64K-entry table. The table
        # values carry a +half-ulp (x1.001953125) compensation so the
        # truncation is unbiased in expectation; residual error is far
        # below the e4m3 quantization itself.
        with np.errstate(invalid="ignore"):
            _STATE["f8_table"] = (
                (
                    np.arange(65536, dtype=np.uint16)
                    .view(ml_dtypes.bfloat16)
                    .astype(np.float32)
                    * np.float32(1.001953125)
                )
                .astype(f8)
                .view(np.uint8)
            )
        _STATE["ok"] = True
    except Exception as e:  # device path unavailable -> numpy fallback
        _STATE["ok"] = False
        _STATE["err"] = e
        return
    try:
        # Exercise the full call path once with synthetic data so the
        # first real call is steady-state (einsum plans, BLAS init,
        # jit dispatch caches).
        rng = np.random.RandomState(0)
        syn = {
            "claim_reps": rng.randn(B, T, D).astype(np.float32),
            "sentence_token_reps": rng.randn(B, S, T, D).astype(np.float32),
            "claim_token_mask": np.ones((B, T), dtype=bool),
            "token_mask": np.ones((B, S, T), dtype=bool),
            "w_sel": rng.randn(KERNEL, 1).astype(np.float32) * 0.02,
            "b_sel": np.zeros((1,), np.float32),
            "w_g1": rng.randn(2 * D, 128).astype(np.float32) * 0.02,
            "b_g1": np.zeros((128,), np.float32),
            "w_g2": rng.randn(128, 1).astype(np.float32) * 0.02,
            "b_g2": np.zeros((1,), np.float32),
            "w_rat": rng.randn(KERNEL, 1).astype(np.float32) * 0.02,
            "b_rat": np.zeros((1,), np.float32),
            "w_lab": rng.randn(2 * D, 3).astype(np.float32) * 0.02,
            "b_lab": np.zeros((3,), np.float32),
        }
        kernel(**syn)
    except Exception:
        pass


# ---------------------------------------------------------------- host tail
def _softmax(x, axis):
    m = np.max(x, axis=axis, keepdims=True)
    e = np.exp(x - m)
    return e / e.sum(axis=axis, keepdims=True)


def _finish_edge(reps, logits, z, hz, w_g1, w_g2, b_g2, w_lab, b_lab):
    """Edge tail: logits (B,S1,S2,T1) -> sentence_label_pred (B,S,3), f32.

    Same math as the reference tail with two BLAS-friendly rewrites:
    z_hat as a batched (i,t)@(t,d) matmul per (b,j), and the gating MLP
    split so the z_exp half (constant over S1) is computed once per
    sentence instead of per (S1,S2) pair.
    """
    t_ = reps.shape[2]
    attn = _softmax(logits, axis=3)                      # softmax over T1
    at = np.ascontiguousarray(attn.transpose(0, 2, 1, 3))        # (b,j,i,t)
    zh = at.reshape(B * S, S, t_) @ reps.reshape(B * S, t_, D)   # (bj,i,d)
    z_hat = np.ascontiguousarray(
        zh.reshape(B, S, S, D).transpose(0, 2, 1, 3)
    )                                                            # (b,i,j,d)
    hh = z_hat.reshape(B * S * S, D) @ w_g1[D:]
    h = np.maximum(hh.reshape(B, S, S, -1) + hz[:, None, :, :], 0)
    beta = _softmax(h @ w_g2 + b_g2, axis=1)             # softmax over S1
    v = np.concatenate([np.sum(beta * z_hat, axis=1), z], axis=-1)
    return _softmax(v @ w_lab + b_lab, axis=-1)


# ------------------------------------------------------------ numpy fallback
def _reference_numpy(claim_reps, reps, token_mask,
                     w_sel, b_sel, w_g1, b_g1, w_g2, b_g2, w_rat, b_rat,
                     w_lab, b_lab):
    reps = reps.astype(np.float64)
    maskf = token_mask.astype(np.float64)
    norms = np.linalg.norm(reps, axis=-1)
    dot = np.einsum("bipd,bjqd->bijpq", reps, reps)
    sim = dot / np.maximum(
        norms[:, :, None, :, None] * norms[:, None, :, None, :], EPS
    )
    rbf = np.exp(-0.5 * ((sim[..., None] - MU) / SIGMA) ** 2)
    pool = rbf.sum(axis=4) * maskf[:, None, :, :, None]
    Ke = np.log(np.clip(pool, CLAMP_MIN, None))
    logits = Ke @ w_sel.astype(np.float64) + b_sel.astype(np.float64)
    m2 = np.broadcast_to(token_mask[:, None, :, :, None], logits.shape)
    lg = np.where(m2, logits, -10000.0)

    attn = _softmax(lg[..., 0], axis=3)
    z_hat = np.einsum("bjtd,bijt->bijd", reps, attn)
    z = reps[:, :, 0, :]
    z_exp = np.broadcast_to(z[:, None, :, :], z_hat.shape)
    hcat = np.concatenate([z_exp, z_hat], axis=-1)
    h = np.maximum(hcat @ w_g1 + b_g1, 0.0)
    beta = _softmax(h @ w_g2 + b_g2, axis=1)
    v = np.concatenate([np.sum(beta * z_hat, axis=1), z], axis=-1)
    slp = _softmax(v @ w_lab + b_lab, axis=-1)

    claim64 = claim_reps.astype(np.float64)
    ncl = np.linalg.norm(claim64, axis=-1)
    dotn = np.einsum("btd,bstd->bst", claim64, reps)
    simn = dotn / np.maximum(ncl[:, None, :] * norms, EPS)
    rbfn = np.exp(-0.5 * ((simn[..., None] - MU) / SIGMA) ** 2)
    pooln = rbfn * maskf[..., None] * float(reps.shape[2])
    phi = np.mean(np.log(np.clip(pooln, CLAMP_MIN, None)), axis=-2)
    rationale = _softmax(phi @ w_rat + b_rat, axis=1)
    return np.sum(slp * rationale, axis=1)


def kernel(**inputs):
    global LAST_RESULTS
    # If the caller passes device-backed jax arrays, fetch them all in ONE
    # relay sync instead of one per np.asarray below. No-op for numpy.
    try:
        if any(hasattr(v, "addressable_shards") for v in inputs.values()):
            import jax

            inputs = jax.device_get(inputs)
    except Exception:
        pass
    claim_reps = np.ascontiguousarray(
        np.asarray(inputs["claim_reps"], dtype=np.float32)
    )
    reps = np.ascontiguousarray(
        np.asarray(inputs["sentence_token_reps"], dtype=np.float32)
    )
    claim_token_mask = np.asarray(inputs["claim_token_mask"])
    token_mask = np.asarray(inputs["token_mask"])
    w_sel = np.asarray(inputs["w_sel"], dtype=np.float32)
    b_sel = np.asarray(inputs["b_sel"], dtype=np.float32)
    w_g1 = np.asarray(inputs["w_g1"], dtype=np.float32)
    b_g1 = np.asarray(inputs["b_g1"], dtype=np.float32)
    w_g2 = np.asarray(inputs["w_g2"], dtype=np.float32)
    b_g2 = np.asarray(inputs["b_g2"], dtype=np.float32)
    w_rat = np.asarray(inputs["w_rat"], dtype=np.float32)
    b_rat = np.asarray(inputs["b_rat"], dtype=np.float32)
    w_lab = np.asarray(inputs["w_lab"], dtype=np.float32)
    b_lab = np.asarray(inputs["b_lab"], dtype=np.float32)

    if not (token_mask.all() and claim_token_mask.all()) or not _STATE.get("ok"):
        out = _reference_numpy(claim_reps, reps, token_mask,
                               w_sel, b_sel, w_g1, b_g1, w_g2, b_g2,
                               w_rat, b_rat, w_lab, b_lab)
        return out.astype(np.float32)

    try:
        return _kernel_device(claim_reps, reps, w_sel, b_sel, w_g1, b_g1,
                              w_g2, b_g2, w_rat, b_rat, w_lab, b_lab)
    except Exception:
        out = _reference_numpy(claim_reps, reps, token_mask,
                               w_sel, b_sel, w_g1, b_g1, w_g2, b_g2,
                               w_rat, b_rat, w_lab, b_lab)
        return out.astype(np.float32)


def _kernel_device(claim_reps, reps, w_sel, b_sel, w_g1, b_g1,
                   w_g2, b_g2, w_rat, b_rat, w_lab, b_lab):
    global LAST_RESULTS
    sharded = _STATE["sharded"]
    f8 = _STATE["f8"]

    # --- host prep: cast raw reps to fp8 straight into the t-major
    # shipping buffer (cast + layout in one gather pass; the D-major
    # transpose happens on the PE via identity matmuls). Normalization
    # happens on device via inv norms shipped in the consts rows.
    RPC = 270  # rows per core: 256 token rows + 14 rows of consts bytes
    g_rp = np.empty((N_CORES * RPC, D), dtype=f8)
    tab8 = _STATE["f8_table"].view(f8)
    hi = reps.view(np.uint16)[..., 1::2]                    # (B,S,T,D)
    norms = np.sqrt(np.einsum("bstd,bstd->bst", reps, reps, optimize=True))
    inv_all = (1.0 / norms).astype(np.float32).reshape(N_CORES * 256)
    wk = np.concatenate(
        [np.tile(w_sel[1:, 0], SG), (-MU32[1:])]
    ).astype(np.float32)
    cblock = np.zeros(14 * D, dtype=np.uint8)
    cblock[: wk.nbytes] = wk.view(np.uint8)
    cblock[1320 + 1024 : 1320 + 1024 + inv_all.nbytes] = inv_all.view(np.uint8)
    for c in range(N_CORES):
        b, ig = divmod(c, 4)
        g_rp[c * RPC : c * RPC + 256, :] = (
            tab8[hi[b, ig * 4 : (ig + 1) * 4].reshape(256, D)]
        )
        cblock[1320 : 1320 + 1024] = (
            inv_all[c * 256 : (c + 1) * 256].view(np.uint8)
        )
        g_rp[c * RPC + 256 : (c + 1) * RPC, :] = cblock.view(f8).reshape(14, D)

    # --- device: edge-kernel logits on cores 0-7 ---
    out = sharded(g_rp)
    try:
        out[0].copy_to_host_async()
    except Exception:
        pass
    LAST_RESULTS = out

    # --- node kernel (independent of device result) overlaps the wait ---
    ncl = np.sqrt(np.einsum("btd,btd->bt", claim_reps, claim_reps))
    dotn = np.einsum("btd,bstd->bst", claim_reps, reps)
    simn = dotn / np.maximum(ncl[:, None, :] * norms, np.float32(EPS))
    rbfn = np.exp(np.float32(-0.5) * ((simn[..., None] - MU32) / SIG32) ** 2)
    pooln = rbfn * np.float32(T)
    phi = np.mean(np.log(np.clip(pooln, np.float32(CLAMP_MIN), None)), axis=-2)
    rationale = _softmax(phi @ w_rat + b_rat, axis=1)
    z = reps[:, :, 0, :]
    hz = z @ w_g1[:D] + b_g1                             # (b,j,128)

    lo_g = np.asarray(out[0]).reshape(N_CORES, 2, 128, SG)

    # --- gather: per-core (2,128,32) -> (B,S1,S2,T1) ---
    logits = np.empty((B, S, S, T), dtype=np.float32)
    for c in range(N_CORES):
        b, ig = divmod(c, 4)
        for ip in range(2):
            for a in range(2):
                i = ig * 4 + ip * 2 + a
                logits[b, i, :, :] = np.transpose(
                    lo_g[c, ip, a * 64 : (a + 1) * 64, b * S : (b + 1) * S]
                )
    logits += b_sel[0]  # constant over T1 (softmax-invariant); keep exactness

    slp = _finish_edge(reps, logits, z, hz, w_g1, w_g2, b_g2, w_lab, b_lab)
    return np.sum(slp * rationale, axis=1).astype(np.float32)


_setup()


# revision 21
# speedup vs baseline: 1.0676x; 1.0676x over previous
"""Trainium2 Bass kernel for nn_KernelGraphAttentionNetwork.

Strategy (8 NeuronCores):
  Ship each core a UNIQUE (256, 768) fp8-e4m3 t-major slice of the RAW
  sentence reps (cores 0-3: batch 0 query-groups 0-3, cores 4-7:
  batch 1), cast on host via a bias-compensated 64K lookup table
  straight into the shipping buffer. The per-core consts rows carry
  the selector weights and the f32 inverse norms as raw bytes
  (bitcast to f32 on device). On device: AllGather over NeuronLink
  gives every core all 2048 columns (both batches); PE identity
  transposes build the D-major lhsT/rhs tiles; then each core
  computes the edge-kernel logits for its own 256 query tokens
  against all 32 gathered sentences:
      sim   = (x_own^T @ x_all) * inv_i * inv_j  (PE + DVE/ScalarE)
      rbf_k = exp(-(sim-mu_k)^2/(2 s_k^2))       (ScalarE Square+Exp)
      pool  = sum_q rbf_k                        (VectorE reduce over T2)
      logit = sum_k ln(clip(pool)) * w_sel       (ScalarE Ln + VectorE)
  Host keeps the 16 columns of the core's own batch and finishes the
  small coupled tail in f32 numpy: T1-softmax, z_hat einsum, gating
  MLP, beta softmax over S1, label head, node kernel, rationale (the
  logits-independent parts run while the device round-trip is in
  flight; the result is fetched via a pre-queued async D2H).

  All heavy one-time setup (jax/concourse import, Bass build, NEFF
  compile, executable load, axon warm-up) happens at module import so
  kernel() itself is a single warm dispatch. The device kernel is
  partition-id free (uniform SPMD program; per-core behavior comes
  only from per-core input data), executed on cores 0-7 through the
  same bass_exec/PJRT path run_bass_kernel_spmd uses under axon.
"""

import numpy as np

KERNEL = 11
B, S, T, D = 2, 16, 64, 768
SG = 2 * S           # 32 gathered sentences (both batches)
EPS = 1e-6
CLAMP_MIN = 1e-6
N_CORES = 8
NK = KERNEL - 1      # k=0 (sigma=1e-3) is constant over T1 -> softmax-invariant


def _kernel_mus(n):
    mus = [1.0]
    if n == 1:
        return mus
    b = 2.0 / (n - 1)
    mus.append(1.0 - b / 2.0)
    for i in range(1, n - 1):
        mus.append(mus[i] - b)
    return mus


MU = np.asarray(_kernel_mus(KERNEL), dtype=np.float64)
SIGMA = np.asarray([0.001] + [0.1] * (KERNEL - 1), dtype=np.float64)
MU32 = MU.astype(np.float32)
SIG32 = SIGMA.astype(np.float32)

_STATE = {}
LAST_RESULTS = None


def _build_sharded():
    import jax
    from jax.sharding import Mesh, PartitionSpec as P
    from concourse.bass2jax import bass_jit, bass_shard_map
    from concourse import mybir, masks
    import concourse.bass as bass
    import concourse.tile as tile

    @bass_jit(trn_type="TRN2", enable_asserts=False, num_devices=N_CORES)
    def _edge_logits_ag(nc, rpart):
        f32 = mybir.dt.float32
        f8 = mybir.dt.float8e4
        AF = mybir.ActivationFunctionType
        bf16 = mybir.dt.bfloat16
        logits_out = nc.dram_tensor(
            "logits_out", [2, 128, SG], bf16, kind="ExternalOutput"
        )
        rpart = rpart[:]

        with tile.TileContext(nc) as tc:
            with (
                tc.tile_pool(name="dram", bufs=1, space="DRAM") as dram_pool,
                tc.tile_pool(name="rt", bufs=1) as rt_pool,
                tc.tile_pool(name="ri", bufs=1) as ri_pool,
                tc.tile_pool(name="stg", bufs=4) as stg_pool,
                tc.tile_pool(name="cst", bufs=1) as cst_pool,
                tc.tile_pool(name="psum", bufs=1, space="PSUM") as psum_pool,
                tc.tile_pool(name="ptr", bufs=4, space="PSUM") as ptr_pool,
                tc.tile_pool(name="work", bufs=4) as work_pool,
                tc.tile_pool(name="pacc", bufs=2) as pacc_pool,
                tc.tile_pool(name="outs", bufs=2) as out_pool,
            ):
                # input is t-major (256 token rows x 768 dims); all-gather
                # the raw slices, then build the D-major lhsT/rhs tiles on
                # the PE via identity transposes (host ships natural layout
                # and skips its 2ms strided transpose pass).
                in_bounce = dram_pool.tile([256, D], f8, tag="inb")
                ag_out = dram_pool.tile([N_CORES * 256, D], f8, tag="agb")
                nc.gpsimd.dma_start(in_bounce[:], rpart[:256, :])
                nc.gpsimd.collective_compute(
                    "AllGather",
                    mybir.AluOpType.bypass,
                    replica_groups=[list(range(N_CORES))],
                    ins=[in_bounce.opt()],
                    outs=[ag_out.opt()],
                )

                ident = cst_pool.tile([128, 128], f8, tag="ident")
                masks.make_identity(nc, ident[:])

                ri = []
                rt = []
                for dc in range(6):
                    ri_t = ri_pool.tile([128, 256], f8, tag=f"ri{dc}")
                    ri.append(ri_t)
                    rt_t = rt_pool.tile([128, SG * T], f8, tag=f"rt{dc}")
                    rt.append(rt_t)
                agp = ag_out[:]

                def _load_transposed(src_rows, col_base, dst_tiles):
                    # src_rows: DRAM AP (128 t-rows, 768); writes column block
                    # [col_base:col_base+128] of each dst_tiles[dc].
                    st = stg_pool.tile([128, D], f8, tag="stage")
                    nc.sync.dma_start(out=st, in_=src_rows)
                    for dc in range(6):
                        # fp8 PE transpose requires output element step 2
                        ptr = ptr_pool.tile([128, 256], f8, tag="tr")
                        pv = ptr[:, 0:256:2]
                        nc.tensor.transpose(
                            pv, st[:, dc * 128 : (dc + 1) * 128], ident[:]
                        )
                        nc.any.tensor_copy(
                            dst_tiles[dc][:, col_base : col_base + 128], pv
                        )

                for half in range(2):
                    _load_transposed(
                        rpart[half * 128 : (half + 1) * 128, :], half * 128, ri
                    )
                for g in range(N_CORES):
                    for half in range(2):
                        _load_transposed(
                            agp[g * 256 + half * 128 : g * 256 + (half + 1) * 128, :],
                            g * 256 + half * 128,
                            rt,
                        )

                # consts ride as raw bytes in rows D..D+42 of rpart
                # (wk f32[330] | inv_own f32[256] | inv_all f32[2048]):
                # broadcast the fp8 bytes to all partitions, then bitcast
                # the SBUF tiles to f32 at the consumers.
                NCONST = SG * NK + NK
                cbase = rpart.offset + 256 * D
                csb = cst_pool.tile([128, 4 * NCONST], f8)
                nc.sync.dma_start(
                    out=csb,
                    in_=bass.AP(tensor=rpart.tensor, offset=cbase,
                                ap=[[0, 128], [1, 4 * NCONST]]),
                )
                cf = csb[:].bitcast(f32)             # (128, NCONST)
                wsel_b = cf[:, : SG * NK]
                negmu_b = cf[:, SG * NK :]
                invi = []                            # own 1/|x| per partition
                for ip in range(2):
                    t_ = cst_pool.tile([128, 4], f8, tag=f"invi{ip}")
                    nc.sync.dma_start(
                        out=t_,
                        in_=bass.AP(tensor=rpart.tensor,
                                    offset=cbase + 4 * NCONST + ip * 512,
                                    ap=[[4, 128], [1, 4]]),
                    )
                    invi.append(t_[:].bitcast(f32))  # (128, 1)
                invj_t = cst_pool.tile([128, 4 * SG * T], f8)
                nc.sync.dma_start(
                    out=invj_t,
                    in_=bass.AP(tensor=rpart.tensor,
                                offset=cbase + 4 * NCONST + 1024,
                                ap=[[0, 128], [1, 4 * SG * T]]),
                )
                invj = invj_t[:].bitcast(f32)        # (128, 2048) bcast

                NCH = (SG * T) // 512
                for ip in range(2):
                    sim_sb = work_pool.tile([128, SG * T], f32, tag="sim_sb")
                    for nch in range(NCH):
                        ps = psum_pool.tile([128, 512], f32, tag=f"sim{nch}")
                        for dc in range(6):
                            nc.tensor.matmul(
                                ps,
                                lhsT=ri[dc][:, ip * 128 : (ip + 1) * 128],
                                rhs=rt[dc][:, nch * 512 : (nch + 1) * 512],
                                start=(dc == 0),
                                stop=(dc == 5),
                            )
                        # fold 1/|x_j| here; 1/|x_i| rides the Square scale
                        nc.vector.tensor_mul(
                            out=sim_sb[:, nch * 512 : (nch + 1) * 512],
                            in0=ps,
                            in1=invj[:, nch * 512 : (nch + 1) * 512],
                        )

                    poolk = pacc_pool.tile([128, SG, NK], f32)
                    for kk in range(NK):
                        k = kk + 1
                        alpha = float(0.5 / (SIGMA[k] ** 2))
                        d2 = work_pool.tile([128, SG * T], f32, tag="d2")
                        nc.scalar.activation(
                            out=d2,
                            in_=sim_sb,
                            func=AF.Square,
                            bias=negmu_b[:, kk : kk + 1],
                            scale=invi[ip],
                        )
                        e = work_pool.tile([128, SG * T], f32, tag="e")
                        nc.scalar.activation(out=e, in_=d2, func=AF.Exp, scale=-alpha)
                        nc.vector.reduce_sum(
                            out=poolk[:, :, kk : kk + 1],
                            in_=e.rearrange("p (j q) -> p j q", q=T),
                            axis=mybir.AxisListType.X,
                        )

                    pkf = poolk.rearrange("p j k -> p (j k)")
                    nc.vector.tensor_scalar_max(out=pkf, in0=pkf, scalar1=CLAMP_MIN)
                    ke = work_pool.tile([128, SG * NK], f32, tag="ke")
                    nc.scalar.activation(out=ke, in_=pkf, func=AF.Ln)
                    nc.vector.tensor_mul(out=ke, in0=ke, in1=wsel_b)
                    lg = out_pool.tile([128, SG], bf16, tag="lg")
                    with nc.allow_low_precision(
                        reason="logits returned bf16; 10-term sum, tol 2e-2"
                    ):
                        nc.vector.reduce_sum(
                            out=lg,
                            in_=ke.rearrange("p (j k) -> p j k", k=NK),
                            axis=mybir.AxisListType.X,
                        )
                    nc.sync.dma_start(out=logits_out[:][ip], in_=lg)
        return (logits_out,)

    devices = jax.devices()[:N_CORES]
    mesh = Mesh(np.asarray(devices), ("core",))
    sharded = bass_shard_map(
        _edge_logits_ag,
        mesh=mesh,
        in_specs=(P("core"),),
        out_specs=(P("core"),),
    )
    return sharded


def _setup():
    try:
        import ml_dtypes

        sharded = _build_sharded()
        f8 = ml_dtypes.float8_e4m3
        dummy_rp = np.zeros((N_CORES * 270, D), f8)
        out = sharded(dummy_rp)
        np.asarray(out[0])  # force compile + load + one execution
        _STATE["sharded"] = sharded
        _STATE["f8"] = f8
        # f32 -> e4m3 via truncate-to-bf16 + 64K-entry table. The table
        # values carry a +half-ulp (x1.001953125) compensation so the
        # truncation is unbiased in expectation; residual error is far
        # below the e4m3 quantization itself.
        with np.errstate(invalid="ignore"):
            _STATE["f8_table"] = (
                (
                    np.arange(65536, dtype=np.uint16)
                    .view(ml_dtypes.bfloat16)
                    .astype(np.float32)
                    * np.float32(1.001953125)
                )
                .astype(f8)
                .view(np.uint8)
            )
        try:
            import ctypes, subprocess, tempfile, textwrap

            csrc = textwrap.dedent("""
                #include <stdint.h>
                void cast_f8(const uint16_t* restrict hi2,
                             uint8_t* restrict out,
                             const uint8_t* restrict table, long n) {
                    for (long i = 0; i < n; i++) out[i] = table[hi2[2*i + 1]];
                }
            """)
            cdir = tempfile.mkdtemp(prefix="castf8_")
            with open(cdir + "/castf8.c", "w") as fobj:
                fobj.write(csrc)
            subprocess.check_call(
                ["gcc", "-O3", "-march=native", "-shared", "-fPIC",
                 "-o", cdir + "/castf8.so", cdir + "/castf8.c"],
                stdout=subprocess.DEVNULL, stderr=subprocess.DEVNULL,
            )
            lib = ctypes.CDLL(cdir + "/castf8.so")
            lib.cast_f8.argtypes = [ctypes.c_void_p] * 3 + [ctypes.c_long]
            lib.cast_f8.restype = None
            _STATE["cast_lib"] = lib
        except Exception:
            pass  # numpy gather fallback
        _STATE["ok"] = True
    except Exception as e:  # device path unavailable -> numpy fallback
        _STATE["ok"] = False
        _STATE["err"] = e
        return
    try:
        # Exercise the full call path once with synthetic data so the
        # first real call is steady-state (einsum plans, BLAS init,
        # jit dispatch caches).
        rng = np.random.RandomState(0)
        syn = {
            "claim_reps": rng.randn(B, T, D).astype(np.float32),
            "sentence_token_reps": rng.randn(B, S, T, D).astype(np.float32),
            "claim_token_mask": np.ones((B, T), dtype=bool),
            "token_mask": np.ones((B, S, T), dtype=bool),
            "w_sel": rng.randn(KERNEL, 1).astype(np.float32) * 0.02,
            "b_sel": np.zeros((1,), np.float32),
            "w_g1": rng.randn(2 * D, 128).astype(np.float32) * 0.02,
            "b_g1": np.zeros((128,), np.float32),
            "w_g2": rng.randn(128, 1).astype(np.float32) * 0.02,
            "b_g2": np.zeros((1,), np.float32),
            "w_rat": rng.randn(KERNEL, 1).astype(np.float32) * 0.02,
            "b_rat": np.zeros((1,), np.float32),
            "w_lab": rng.randn(2 * D, 3).astype(np.float32) * 0.02,
            "b_lab": np.zeros((3,), np.float32),
        }
        kernel(**syn)
    except Exception:
        pass


# ---------------------------------------------------------------- host tail
def _softmax(x, axis):
    m = np.max(x, axis=axis, keepdims=True)
    e = np.exp(x - m)
    return e / e.sum(axis=axis, keepdims=True)


def _finish_edge(reps, logits, z, hz, w_g1, w_g2, b_g2, w_lab, b_lab):
    """Edge tail: logits (B,S1,S2,T1) -> sentence_label_pred (B,S,3), f32.

    Same math as the reference tail with two BLAS-friendly rewrites:
    z_hat as a batched (i,t)@(t,d) matmul per (b,j), and the gating MLP
    split so the z_exp half (constant over S1) is computed once per
    sentence instead of per (S1,S2) pair.
    """
    t_ = reps.shape[2]
    attn = _softmax(logits, axis=3)                      # softmax over T1
    at = np.ascontiguousarray(attn.transpose(0, 2, 1, 3))        # (b,j,i,t)
    zh = at.reshape(B * S, S, t_) @ reps.reshape(B * S, t_, D)   # (bj,i,d)
    z_hat = np.ascontiguousarray(
        zh.reshape(B, S, S, D).transpose(0, 2, 1, 3)
    )                                                            # (b,i,j,d)
    hh = z_hat.reshape(B * S * S, D) @ w_g1[D:]
    h = np.maximum(hh.reshape(B, S, S, -1) + hz[:, None, :, :], 0)
    beta = _softmax(h @ w_g2 + b_g2, axis=1)             # softmax over S1
    v = np.concatenate([np.sum(beta * z_hat, axis=1), z], axis=-1)
    return _softmax(v @ w_lab + b_lab, axis=-1)


# ------------------------------------------------------------ numpy fallback
def _reference_numpy(claim_reps, reps, token_mask,
                     w_sel, b_sel, w_g1, b_g1, w_g2, b_g2, w_rat, b_rat,
                     w_lab, b_lab):
    reps = reps.astype(np.float64)
    maskf = token_mask.astype(np.float64)
    norms = np.linalg.norm(reps, axis=-1)
    dot = np.einsum("bipd,bjqd->bijpq", reps, reps)
    sim = dot / np.maximum(
        norms[:, :, None, :, None] * norms[:, None, :, None, :], EPS
    )
    rbf = np.exp(-0.5 * ((sim[..., None] - MU) / SIGMA) ** 2)
    pool = rbf.sum(axis=4) * maskf[:, None, :, :, None]
    Ke = np.log(np.clip(pool, CLAMP_MIN, None))
    logits = Ke @ w_sel.astype(np.float64) + b_sel.astype(np.float64)
    m2 = np.broadcast_to(token_mask[:, None, :, :, None], logits.shape)
    lg = np.where(m2, logits, -10000.0)

    attn = _softmax(lg[..., 0], axis=3)
    z_hat = np.einsum("bjtd,bijt->bijd", reps, attn)
    z = reps[:, :, 0, :]
    z_exp = np.broadcast_to(z[:, None, :, :], z_hat.shape)
    hcat = np.concatenate([z_exp, z_hat], axis=-1)
    h = np.maximum(hcat @ w_g1 + b_g1, 0.0)
    beta = _softmax(h @ w_g2 + b_g2, axis=1)
    v = np.concatenate([np.sum(beta * z_hat, axis=1), z], axis=-1)
    slp = _softmax(v @ w_lab + b_lab, axis=-1)

    claim64 = claim_reps.astype(np.float64)
    ncl = np.linalg.norm(claim64, axis=-1)
    dotn = np.einsum("btd,bstd->bst", claim64, reps)
    simn = dotn / np.maximum(ncl[:, None, :] * norms, EPS)
    rbfn = np.exp(-0.5 * ((simn[..., None] - MU) / SIGMA) ** 2)
    pooln = rbfn * maskf[..., None] * float(reps.shape[2])
    phi = np.mean(np.log(np.clip(pooln, CLAMP_MIN, None)), axis=-2)
    rationale = _softmax(phi @ w_rat + b_rat, axis=1)
    return np.sum(slp * rationale, axis=1)


def kernel(**inputs):
    global LAST_RESULTS
    # If the caller passes device-backed jax arrays, fetch them all in ONE
    # relay sync instead of one per np.asarray below. No-op for numpy.
    try:
        if any(hasattr(v, "addressable_shards") for v in inputs.values()):
            import jax

            inputs = jax.device_get(inputs)
    except Exception:
        pass
    claim_reps = np.ascontiguousarray(
        np.asarray(inputs["claim_reps"], dtype=np.float32)
    )
    reps = np.ascontiguousarray(
        np.asarray(inputs["sentence_token_reps"], dtype=np.float32)
    )
    claim_token_mask = np.asarray(inputs["claim_token_mask"])
    token_mask = np.asarray(inputs["token_mask"])
    w_sel = np.asarray(inputs["w_sel"], dtype=np.float32)
    b_sel = np.asarray(inputs["b_sel"], dtype=np.float32)
    w_g1 = np.asarray(inputs["w_g1"], dtype=np.float32)
    b_g1 = np.asarray(inputs["b_g1"], dtype=np.float32)
    w_g2 = np.asarray(inputs["w_g2"], dtype=np.float32)
    b_g2 = np.asarray(inputs["b_g2"], dtype=np.float32)
    w_rat = np.asarray(inputs["w_rat"], dtype=np.float32)
    b_rat = np.asarray(inputs["b_rat"], dtype=np.float32)
    w_lab = np.asarray(inputs["w_lab"], dtype=np.float32)
    b_lab = np.asarray(inputs["b_lab"], dtype=np.float32)

    if not (token_mask.all() and claim_token_mask.all()) or not _STATE.get("ok"):
        out = _reference_numpy(claim_reps, reps, token_mask,
                               w_sel, b_sel, w_g1, b_g1, w_g2, b_g2,
                               w_rat, b_rat, w_lab, b_lab)
        return out.astype(np.float32)

    try:
        return _kernel_device(claim_reps, reps, w_sel, b_sel, w_g1, b_g1,
                              w_g2, b_g2, w_rat, b_rat, w_lab, b_lab)
    except Exception:
        out = _reference_numpy(claim_reps, reps, token_mask,
                               w_sel, b_sel, w_g1, b_g1, w_g2, b_g2,
                               w_rat, b_rat, w_lab, b_lab)
        return out.astype(np.float32)


def _kernel_device(claim_reps, reps, w_sel, b_sel, w_g1, b_g1,
                   w_g2, b_g2, w_rat, b_rat, w_lab, b_lab):
    global LAST_RESULTS
    sharded = _STATE["sharded"]
    f8 = _STATE["f8"]

    # --- host prep: cast raw reps to fp8 straight into the t-major
    # shipping buffer (cast + layout in one gather pass; the D-major
    # transpose happens on the PE via identity matmuls). Normalization
    # happens on device via inv norms shipped in the consts rows.
    RPC = 270  # rows per core: 256 token rows + 14 rows of consts bytes
    g_rp = np.empty((N_CORES * RPC, D), dtype=f8)
    tab8 = _STATE["f8_table"].view(f8)
    hi = reps.view(np.uint16)[..., 1::2]                    # (B,S,T,D)
    norms = np.sqrt(np.einsum("bstd,bstd->bst", reps, reps, optimize=True))
    inv_all = (1.0 / norms).astype(np.float32).reshape(N_CORES * 256)
    wk = np.concatenate(
        [np.tile(w_sel[1:, 0], SG), (-MU32[1:])]
    ).astype(np.float32)
    cblock = np.zeros(14 * D, dtype=np.uint8)
    cblock[: wk.nbytes] = wk.view(np.uint8)
    cblock[1320 + 1024 : 1320 + 1024 + inv_all.nbytes] = inv_all.view(np.uint8)
    cast_lib = _STATE.get("cast_lib")
    tbl_ptr = _STATE["f8_table"].ctypes.data
    for c in range(N_CORES):
        b, ig = divmod(c, 4)
        dst = g_rp[c * RPC : c * RPC + 256, :]
        if cast_lib is not None:
            src_blk = reps[b, ig * 4 : (ig + 1) * 4]
            cast_lib.cast_f8(src_blk.ctypes.data, dst.ctypes.data,
                             tbl_ptr, 256 * D)
        else:
            dst[:] = tab8[hi[b, ig * 4 : (ig + 1) * 4].reshape(256, D)]
        cblock[1320 : 1320 + 1024] = (
            inv_all[c * 256 : (c + 1) * 256].view(np.uint8)
        )
        g_rp[c * RPC + 256 : (c + 1) * RPC, :] = cblock.view(f8).reshape(14, D)

    # --- device: edge-kernel logits on cores 0-7 ---
    out = sharded(g_rp)
    try:
        out[0].copy_to_host_async()
    except Exception:
        pass
    LAST_RESULTS = out

    # --- node kernel (independent of device result) overlaps the wait ---
    ncl = np.sqrt(np.einsum("btd,btd->bt", claim_reps, claim_reps))
    dotn = np.einsum("btd,bstd->bst", claim_reps, reps)
    simn = dotn / np.maximum(ncl[:, None, :] * norms, np.float32(EPS))
    rbfn = np.exp(np.float32(-0.5) * ((simn[..., None] - MU32) / SIG32) ** 2)
    pooln = rbfn * np.float32(T)
    phi = np.mean(np.log(np.clip(pooln, np.float32(CLAMP_MIN), None)), axis=-2)
    rationale = _softmax(phi @ w_rat + b_rat, axis=1)
    z = reps[:, :, 0, :]
    hz = z @ w_g1[:D] + b_g1                             # (b,j,128)

    lo_g = np.asarray(out[0]).reshape(N_CORES, 2, 128, SG)

    # --- gather: per-core (2,128,32) -> (B,S1,S2,T1) ---
    logits = np.empty((B, S, S, T), dtype=np.float32)
    for c in range(N_CORES):
        b, ig = divmod(c, 4)
        for ip in range(2):
            for a in range(2):
                i = ig * 4 + ip * 2 + a
                logits[b, i, :, :] = np.transpose(
                    lo_g[c, ip, a * 64 : (a + 1) * 64, b * S : (b + 1) * S]
                )
    logits += b_sel[0]  # constant over T1 (softmax-invariant); keep exactness

    slp = _finish_edge(reps, logits, z, hz, w_g1, w_g2, b_g2, w_lab, b_lab)
    return np.sum(slp * rationale, axis=1).astype(np.float32)


_setup()
